# revision 40
# baseline (speedup 1.0000x reference)
"""Trainium2 Bass kernel for multi-lengthscale RBF kernel self-attention.

Reference computation (B=2, N=4096, D=128, 4 heads of 32):
  d2[b,i,j] = ||coords[b,i]-coords[b,j]||^2
  att_h = exp(-d2/ls_h^2) row-normalized (+1e-8), ls = [0.5,1,2,4]
  out = concat_h(att_h @ (features @ Wv[h] + bv[h])) @ Wo + bo

Device strategy (8 cores, query rows sharded, 512 queries/core/batch):
  * Gram trick with SPLIT-BF16 precision: -d2[j,i] = 2 xj.xi - |xj|^2
    - |xi|^2 as ONE K=13 bf16 matmul per (batch, j-block): every fp32
    input value is decomposed hi+lo into two bf16 K-rows, so products
    keep ~16-bit accuracy while streaming at 1 cycle/row (plain fp32
    matmul costs 4 cycles/row; this is the main win over the fp32
    baseline, final rel err ~3.4e-3 vs the 2e-2 gate).
  * j-blocks processed in PAIRS: one [128,1024] PSUM gram tile (double
    buffered, 2x2 banks) feeds [128,1024]-wide ACT/DVE ops, amortizing
    per-op fixed overheads. PSUM: att 4 banks + g 2x2 banks = 8; the
    Wo-projection accumulator borrows a g-pool slot.
  * e1=exp(G/16) [ls=4], e3=exp(G) [ls=1] on ACT (bf16 out);
    e4=(e3^2)^2 on DVE; e2 from exp(G/4) on ACT or (e1^2)^2 on DVE per
    the measured-cost pattern E2PAT (gpsimd muls are slower and raise
    the power throttle, so they are avoided).
  * att_h @ V_h with bf16 V (+ ones column) stationary [128,33];
    PSUM att [33, 4*512]: rows 0..31 = head outputs^T, row 32 = rowsums.
    att matmuls are emitted LAG pairs behind the gram/elementwise ops
    (software pipelining) so the PE never waits on the ACT/DVE chain,
    and each batch's epilogue is emitted after the next batch's first
    pairs so it overlaps their streaming.
  * Epilogue, phase-ordered across engines: rowsum row -> SBUF
    (ACT/DVE), reciprocal_approx_fast (DVE, needs base partition 0),
    gpsimd partition_broadcast down 32 partitions, DVE normalize to
    bf16, then 4 accumulating K=32 bf16 matmuls against Wo.
  * Output stored [o, i] (transposed); host fixes layout + adds bo_eff.

Host does only O(N*D) marshalling: coord hi/lo augmentation, V = F@Wv
(+ones) in bf16, bo_eff = bo + bv@Wo added at the end, final transpose.
Measured: 420us (fp32 baseline) -> 116-144us depending on the board's
power/thermal state (all engines' clocks drift together ~20%).
"""

import numpy as np

B = 2
N = 4096
NCORES = 8
NQ = N // NCORES          # 512 query rows per core per batch
P = 128                   # partitions / j-block size
NJB = N // P              # 32 j-blocks
NPAIR = NJB // 2          # 16 j-block pairs
KG = 13                   # gram contraction rows (split-bf16)
VW = 33                   # V columns per head incl. ones column
VROW = 4 * VW             # 132 cols per j-block in vall
D = 128

_BUILT = {}


def _build():
    import concourse.bass as bass
    import concourse.bacc as bacc
    import concourse.mybir as mybir
    import concourse.tile as tile

    f32 = mybir.dt.float32
    f32r = mybir.dt.float32r
    bf16 = mybir.dt.bfloat16
    AF = mybir.ActivationFunctionType

    nc = bacc.Bacc("TRN2", target_bir_lowering=False, debug=False,
                   enable_asserts=True, num_devices=NCORES)

    grama = nc.dram_tensor("grama", (B, KG, N), bf16, kind="ExternalInput").ap()
    gramr = nc.dram_tensor("gramr", (B, KG, NQ), bf16, kind="ExternalInput").ap()
    vall_d = nc.dram_tensor("vall", (B, P, NJB * VROW), bf16,
                            kind="ExternalInput").ap()
    wo4_d = nc.dram_tensor("wo4", (32, 4 * D), bf16, kind="ExternalInput").ap()
    outt = nc.dram_tensor("outt", (B, D, NQ), f32, kind="ExternalOutput").ap()

    with tile.TileContext(nc) as tc:
        with (
            tc.tile_pool(name="const", bufs=1) as cp,
            tc.tile_pool(name="elem", bufs=4) as ep,
            tc.tile_pool(name="mnp", bufs=1) as lp,
            tc.tile_pool(name="rsp", bufs=2) as rp,
            tc.tile_pool(name="outp", bufs=2) as op_,
            tc.tile_pool(name="gps", bufs=2, space="PSUM") as gp,
            tc.tile_pool(name="aps", bufs=1, space="PSUM") as ap_,
        ):
            ga = {}
            gr = {}
            va = {}
            GACH = 4
            VACH = 8
            for b in range(B):
                gr[b] = cp.tile([KG, NQ], bf16, tag=f"gr{b}", name=f"gr{b}")
                ga[b] = cp.tile([KG, N], bf16, tag=f"ga{b}", name=f"ga{b}")
                va[b] = cp.tile([P, NJB * VROW], bf16, tag=f"va{b}",
                                name=f"va{b}")
            wo4_sb = cp.tile([32, 4 * D], bf16, tag="wo4")
            # interleave big loads so early j-blocks arrive first
            wg = N // GACH
            wv = NJB * VROW // VACH
            nc.sync.dma_start(gr[0][:], gramr[0])
            nc.sync.dma_start(ga[0][:, 0:wg], grama[0][:, 0:wg])
            nc.sync.dma_start(wo4_sb[:], wo4_d)
            nc.sync.dma_start(gr[1][:], gramr[1])
            for c in range(VACH):
                nc.sync.dma_start(va[0][:, c * wv:(c + 1) * wv],
                                  vall_d[0][:, c * wv:(c + 1) * wv])
                if 0 < c < GACH:
                    nc.sync.dma_start(ga[0][:, c * wg:(c + 1) * wg],
                                      grama[0][:, c * wg:(c + 1) * wg])
            for c in range(GACH):
                nc.sync.dma_start(ga[1][:, c * wg:(c + 1) * wg],
                                  grama[1][:, c * wg:(c + 1) * wg])
            for c in range(VACH):
                nc.sync.dma_start(va[1][:, c * wv:(c + 1) * wv],
                                  vall_d[1][:, c * wv:(c + 1) * wv])

            # e2 producer pattern per pair: ACT direct exp / Pool squares /
            # DVE squares, weighted by measured per-op HW costs.
            E2PAT = "ADADADAD"
            E4PAT = "DDDDDDDD"

            def emit_pair(b, att, pi):
                jb0 = 2 * pi
                g = gp.tile([P, 2 * NQ], f32, tag="g")
                for half in range(2):
                    jb = jb0 + half
                    nc.tensor.matmul(g[:, NQ * half:NQ * (half + 1)],
                                     ga[b][:, P * jb:P * (jb + 1)],
                                     gr[b][:], start=True, stop=True)
                e1 = ep.tile([P, 2 * NQ], bf16, tag="e1")
                nc.scalar.activation(e1[:], g[:], AF.Exp, scale=1.0 / 16.0)
                e3 = ep.tile([P, 2 * NQ], bf16, tag="e3")
                nc.scalar.activation(e3[:], g[:], AF.Exp, scale=1.0)
                e2 = ep.tile([P, 2 * NQ], bf16, tag="e2")
                kind = E2PAT[pi % len(E2PAT)]
                if pi >= NPAIR - 2:
                    kind = "A" if pi == NPAIR - 1 else "D"
                if kind == "A":
                    nc.scalar.activation(e2[:], g[:], AF.Exp, scale=0.25)
                else:
                    eng = nc.vector if kind == "D" else nc.gpsimd
                    t2 = ep.tile([P, 2 * NQ], bf16, tag="t2")
                    eng.tensor_mul(t2[:], e1[:], e1[:])
                    eng.tensor_mul(e2[:], t2[:], t2[:])
                # e4: direct ACT exp on alternate pairs, else DVE squares
                e4 = ep.tile([P, 2 * NQ], bf16, tag="e4")
                k4 = E4PAT[pi % len(E4PAT)]
                if k4 == "A":
                    nc.scalar.activation(e4[:], g[:], AF.Exp, scale=4.0)
                else:
                    eng4 = nc.vector if k4 == "D" else nc.gpsimd
                    t4 = ep.tile([P, 2 * NQ], bf16, tag="t4")
                    eng4.tensor_mul(t4[:], e3[:], e3[:])
                    eng4.tensor_mul(e4[:], t4[:], t4[:])
                return {0: e4, 1: e3, 2: e2, 3: e1}

            def emit_att(b, att, pi, wmap):
                for half in range(2):
                    jb = 2 * pi + half
                    for h in (3, 1, 0, 2):       # in e-readiness order
                        nc.tensor.matmul(
                            att[:, NQ * h:NQ * (h + 1)],
                            va[b][:, VROW * jb + VW * h:
                                  VROW * jb + VW * h + VW],
                            wmap[h][:, NQ * half:NQ * (half + 1)],
                            start=(jb == 0), stop=(jb == NJB - 1))

            def emit_epilogue(b, att, last):
                # Normalize + project + store, phase-ordered so the four
                # per-head chains pipeline across ACT/DVE/Pool instead of
                # serializing: all rowsum stages, then reciprocals, then
                # broadcasts + normalizes, then the Wo projection.
                mn = lp.tile([32, 4 * NQ], bf16, tag="mn")
                s1s, rsrs, rbss = [], [], []
                for h in range(4):
                    sl = slice(NQ * h, NQ * (h + 1))
                    s1 = rp.tile([1, NQ], f32, tag=f"s1_{h}",
                                 name=f"s1_{b}{h}")
                    if h % 2 == 0:
                        nc.scalar.activation(s1[:], att[32:33, sl], AF.Copy)
                    else:
                        nc.vector.tensor_copy(s1[:], att[32:33, sl])
                    s1s.append(s1)
                for h in range(4):
                    rsr = rp.tile([1, NQ], f32, tag=f"rsr_{h}",
                                  name=f"rsr_{b}{h}")
                    nc.vector.reciprocal_approx_fast(rsr[:], s1s[h][:])
                    rsrs.append(rsr)
                for h in range(4):
                    sl = slice(NQ * h, NQ * (h + 1))
                    rbs = rp.tile([32, NQ], f32, tag=f"rbs_{h}",
                                  name=f"rbs_{b}{h}")
                    nc.gpsimd.partition_broadcast(rbs[:], rsrs[h][:])
                    nc.vector.tensor_mul(mn[:, sl], att[0:32, sl], rbs[:])
                po = gp.tile([P, 2 * NQ], f32, tag="g", name=f"po{b}")
                for h in range(4):
                    sl = slice(NQ * h, NQ * (h + 1))
                    nc.tensor.matmul(po[:, 0:NQ], wo4_sb[:, D * h:D * (h + 1)],
                                     mn[:, sl], start=(h == 0), stop=(h == 3))
                osb = op_.tile([D, NQ], f32, tag="osb")
                nc.scalar.activation(osb[:], po[:, 0:NQ], AF.Copy)
                nc.sync.dma_start(outt[b], osb[:])

            # ---- main loops, software-pipelined: att matmuls for pair k
            # are emitted two pairs late so the PE never sits directly
            # behind the ACT/DVE/Pool chain; batch b's epilogue is emitted
            # after batch b+1's first pairs for the same reason.
            LAG = 2
            atts = {}
            pend = []     # (b, pi, wmap) awaiting att emission
            todo_epi = []  # batch whose epilogue is pending

            def drain_one_att():
                bb, pp_, wm = pend.pop(0)
                emit_att(bb, atts[bb], pp_, wm)
                if pp_ == NPAIR - 1:
                    todo_epi.append(bb)

            for b in range(B):
                atts[b] = ap_.tile([VW, 4 * NQ], f32, tag="att",
                                   name=f"att{b}")
                for pi in range(NPAIR):
                    pend.append((b, pi, emit_pair(b, atts[b], pi)))
                    if len(pend) > LAG:
                        drain_one_att()
                    # delay previous batch's epilogue until this batch's
                    # pipeline is primed
                    if pi == 1 and todo_epi:
                        bb = todo_epi.pop(0)
                        emit_epilogue(bb, atts[bb], last=False)
            while pend:
                drain_one_att()
            while todo_epi:
                bb = todo_epi.pop(0)
                emit_epilogue(bb, atts[bb], last=True)

    nc.compile()
    return nc


def _bf16(x):
    import ml_dtypes
    return np.asarray(x, np.float32).astype(ml_dtypes.bfloat16)


def _prep(features, coords, Wv, bv, Wo, bo):
    import ml_dtypes
    coords = np.asarray(coords, np.float32)
    features = np.asarray(features, np.float32)
    Wv = np.asarray(Wv, np.float32)
    bv = np.asarray(bv, np.float32)
    Wo = np.asarray(Wo, np.float32)
    bo = np.asarray(bo, np.float32)

    def split(x):
        hi = _bf16(x).astype(np.float32)
        lo = _bf16(x - hi).astype(np.float32)
        return hi, lo

    # G[j,i] = 2 xi.xj - |xj|^2 - |xi|^2 via 13 bf16 K-rows:
    #   3 coords x (hi_j*hi_i, hi_j*lo_i, lo_j*hi_i) + |xj|^2 hi/lo vs ones
    #   + ones vs |xi|^2 hi/lo
    sq = (coords ** 2).sum(-1)
    one = np.ones_like(sq)
    rows_a, rows_r = [], []
    for c in range(3):
        ah, al = split(coords[..., c])
        bh, bl = split(2.0 * coords[..., c])
        rows_a += [ah, ah, al]
        rows_r += [bh, bl, bh]
    sh, sl_ = split(-sq)
    rows_a += [sh, sl_, one, one]
    rows_r += [one, one, sh, sl_]
    grama = _bf16(np.stack(rows_a, axis=1))            # [B,13,N]
    gramr = _bf16(np.stack(rows_r, axis=1))            # [B,13,N]

    # V (no bv: folded into bo_eff) with ones column per head, laid out
    # [B, 128, NJB*132]: col jb*132 + h*33 + k = V[b, jb*128+p, h, k]
    v = np.einsum('bnd,hdk->bnhk', features, Wv)       # [B, N, 4, 32]
    vaug = np.concatenate([v, np.ones((B, N, 4, 1), np.float32)], axis=-1)
    vall = vaug.reshape(B, NJB, P, VROW).transpose(0, 2, 1, 3).reshape(
        B, P, NJB * VROW)
    vall = _bf16(np.ascontiguousarray(vall))

    # wo4[k, 128h+o] = Wo[32h+k, o] for the four K=32 projection matmuls
    wo4 = _bf16(np.ascontiguousarray(
        Wo.reshape(4, 32, D).transpose(1, 0, 2).reshape(32, 4 * D)))

    bo_eff = bo + bv.reshape(-1) @ Wo                  # [128]
    return grama, gramr, vall, wo4, bo_eff


def kernel(features, coords, Wv, bv, Wo, bo):
    from concourse import bass_utils

    grama, gramr, vall, wo4, bo_eff = _prep(
        features, coords, Wv, bv, Wo, bo)

    if "nc" not in _BUILT:
        _BUILT["nc"] = _build()
    nc = _BUILT["nc"]

    in_maps = []
    for c in range(NCORES):
        sl = slice(c * NQ, (c + 1) * NQ)
        in_maps.append({
            "grama": grama,
            "gramr": np.ascontiguousarray(gramr[:, :, sl]),
            "vall": vall,
            "wo4": wo4,
        })
    res = bass_utils.run_bass_kernel_spmd(nc, in_maps,
                                          core_ids=list(range(NCORES)),
                                          trace=_BUILT.get("trace", False),
                                          tmpdir=_BUILT.get("tmpdir"))
    _BUILT["last_results"] = res

    out = np.empty((B, N, D), np.float32)
    for c in range(NCORES):
        ot = res.results[c]["outt"]                    # [B, 128, 512]
        for b in range(B):
            out[b, c * NQ:(c + 1) * NQ, :] = ot[b].T
    out += bo_eff[None, None, :]
    return out
